# revision 25
# baseline (speedup 1.0000x reference)
"""TRN2 Bass kernel for nn_Attention_43963285242501.

Sharding: 8 cores = (batch b in {0,1}) x (kv-head group g in {0..3}).
Each core computes, for its batch, the 8 query heads + 1 kv head of group g,
the matching 512-wide slices of the gate and of Wo's rows, producing a
partial [L, D] output; the host sums the 4 partials per batch (the
"all-reduce after o_proj" done at unshard time).

V1 redesign vs the 500us baseline:
  - q AND kv projected in natural layout per 128-pos tile (no kvT
    transpose dance); rms-scale via one ACT Rsqrt per tile; rope reads
    PSUM directly and writes fp16; all PE transposes in fp16 (1c/row).
  - qT/kT stored as 64-partition tiles (K=64 matmuls - no zero padding).
  - gate pre-activations stored raw; ALL sigmoids batched once at end of
    phase A -> 3 ACT table loads total (rsqrt/sigmoid/exp).
  - phase C: per-ktile pipeline ST(pair-packed psum) -> exp -> PV
    interleaved, st ring double-buffered; causal diagonal blocks trimmed
    (matmul/exp/mask/PV free sizes shrink by the masked prefix).
  - weights DMA'd on the gpsimd queue, xt on sync, tables on scalar ->
    first matmul starts ~2us in; y stored fp16, host sums in fp32.
"""

import sys

sys.path.insert(0, "/opt/trn_rl_repo")

import numpy as np

import concourse.mybir as mybir
import concourse.tile as tile
from concourse import bacc
from concourse.bass_utils import run_bass_kernel_spmd
from concourse.masks import make_identity

F32 = mybir.dt.float32
FP16 = mybir.dt.float16

B, L, D = 2, 2048, 2048
H, HKV, HD = 32, 4, 64
NH = H // HKV            # q heads per core = 8
NPAIR = NH // 2          # head pairs = 4
P = 128
EPS = 1e-5
THETA = 10000.0
SCALE = HD ** -0.5


def build_core_kernel(Lk=L, Dk=D):
    LT = Lk // P         # pos tiles
    KC = Dk // P         # contraction chunks over D
    QC = Lk // 512       # 512-wide pos chunks
    KT_PER_QC = 512 // P  # 4 pos-tiles per chunk

    nc = bacc.Bacc()
    xt = nc.dram_tensor("xt", [Dk, Lk], FP16, kind="ExternalInput")
    wq = nc.dram_tensor("wq", [Dk, NH * HD], FP16, kind="ExternalInput")
    wkv = nc.dram_tensor("wkv", [Dk, 2 * HD], FP16, kind="ExternalInput")
    wg = nc.dram_tensor("wg", [Dk, NH * HD], FP16, kind="ExternalInput")
    wo = nc.dram_tensor("wo", [NH * HD, Dk], FP16, kind="ExternalInput")
    cos_d = nc.dram_tensor("cos", [Lk, HD // 2], F32, kind="ExternalInput")
    sin_d = nc.dram_tensor("sin", [Lk, HD // 2], F32, kind="ExternalInput")
    mask_d = nc.dram_tensor("mask", [P, P], FP16, kind="ExternalInput")
    y = nc.dram_tensor("y", [Lk, Dk], FP16, kind="ExternalOutput")

    xt_r = xt.rearrange("(ko ki) l -> ki ko l", ki=P)          # [128, KC, Lk]
    wq_r = wq.rearrange("(ko ki) m -> ki ko m", ki=P)          # [128, KC, 512]
    wkv_r = wkv.rearrange("(ko ki) m -> ki ko m", ki=P)        # [128, KC, 128]
    wg_r = wg.rearrange("(ko ki) m -> ki ko m", ki=P)
    wo_r = wo.rearrange("(jo ji) d -> ji jo d", ji=P)          # [128, 4, Dk]
    cos_r = cos_d.rearrange("(t p) c -> p t c", p=P)           # [128, LT, 32]
    sin_r = sin_d.rearrange("(t p) c -> p t c", p=P)
    y_r = y.rearrange("(t p) d -> p t d", p=P)                 # [128, LT, Dk]

    with tile.TileContext(nc) as tc:
        with (
            tc.tile_pool(name="persist", bufs=1) as persist,
            tc.tile_pool(name="consts", bufs=1) as consts,
        ):
            # persistent SBUF
            qkT = persist.tile([HD, NH + 1, Lk], FP16)  # heads 0..7 = qT, 8 = kT
            v_sb = persist.tile([P, LT, P], FP16)       # v | ones | zero-pad
            gateT = persist.tile([P, NPAIR, Lk], FP16)  # post-sigmoid
            wo_sb = persist.tile([P, NH * HD // P, Dk], FP16)

            cs_sb = consts.tile([P, LT, HD], F32)
            sc_sb = consts.tile([P, LT, HD], F32)
            mask_sb = consts.tile([P, P], FP16)
            identh = consts.tile([P, P], FP16)
            eps_sb = consts.tile([P, 1], F32)

            # tables + weights on the gpsimd DMA queue (Pool engine is idle;
            # keeps the big DGE setups off ACT), priority order
            make_identity(nc, identh[:])
            nc.vector.memset(eps_sb[:], EPS)
            nc.vector.memset(v_sb[:], 0.0)
            nc.vector.memset(v_sb[:, :, HD : HD + 1], 1.0)

            # ------- phase A: q/kv/gate projections, norm+rope, transposes ----
            with (
                tc.tile_pool(name="wa", bufs=1) as wa,
                tc.tile_pool(name="xq", bufs=2) as xq_pool,
                tc.tile_pool(name="worka", bufs=2) as worka,
                tc.tile_pool(name="gr", bufs=1) as gr,
                tc.tile_pool(name="psA", bufs=2, space="PSUM") as psA,
                tc.tile_pool(name="psKV", bufs=1, space="PSUM") as psKV,
                tc.tile_pool(name="psN", bufs=1, space="PSUM") as psN,
                tc.tile_pool(name="psG", bufs=2, space="PSUM") as psG,
                tc.tile_pool(name="psT", bufs=1, space="PSUM") as psT,
            ):
                wq_sb = wa.tile([P, KC, NH * HD], FP16)
                wkv_sb = wa.tile([P, KC, 2 * HD], FP16)
                wg_sb = wa.tile([P, KC, NH * HD], FP16)
                graw = gr.tile([P, NPAIR, Lk], FP16)
                nc.gpsimd.dma_start(wkv_sb[:], wkv_r[:, :])
                nc.gpsimd.dma_start(wq_sb[:], wq_r[:, :])
                nc.gpsimd.dma_start(cs_sb.rearrange("p t (h c) -> p t h c", h=2)[:, :, 0], cos_r)
                nc.gpsimd.dma_start(cs_sb.rearrange("p t (h c) -> p t h c", h=2)[:, :, 1], sin_r)
                nc.gpsimd.dma_start(sc_sb.rearrange("p t (h c) -> p t h c", h=2)[:, :, 0], sin_r)
                nc.gpsimd.dma_start(sc_sb.rearrange("p t (h c) -> p t h c", h=2)[:, :, 1], cos_r)
                nc.gpsimd.dma_start(mask_sb[:], mask_d[:, :])
                nc.gpsimd.dma_start(wg_sb[:], wg_r[:, :])
                nc.gpsimd.dma_start(wo_sb[:], wo_r[:, :])

                def trans_block(pt, qro, kro):
                    """PE transposes of one tile's roped q heads + k, then one
                    bulk copy into qkT. Issued one tile late so the PE never
                    waits on the DVE norm/rope chain."""
                    psl = slice(pt * P, (pt + 1) * P)
                    trq = psT.tile([HD, NH + 1, P], FP16, tag="trq")
                    for h in range(NH):
                        nc.tensor.transpose(trq[:, h], qro[:, h, :], identh[:])
                    nc.tensor.transpose(trq[:, NH], kro[:], identh[:])
                    nc.scalar.copy(out=qkT[:, :, psl], in_=trq[:])

                pending = None
                for qtr in range(QC):
                    xt_q = xq_pool.tile([P, KC, 512], FP16, tag="xtq")
                    nc.sync.dma_start(
                        xt_q[:], xt_r[:, :, qtr * 512 : (qtr + 1) * 512]
                    )

                    # kvT for the whole quarter: [128 (k|v dims), 512 pos]
                    kv_ps = psKV.tile([P, 512], F32, tag="kvps")
                    for kc in range(KC):
                        nc.tensor.matmul(
                            kv_ps[:],
                            wkv_sb[:, kc],
                            xt_q[:, kc],
                            start=(kc == 0),
                            stop=(kc == KC - 1),
                        )
                    kvT_f = worka.tile([P, 512], FP16, tag="kvtf")
                    nc.vector.tensor_copy(kvT_f[:], kv_ps[:])

                    for t in range(KT_PER_QC):
                        pt = qtr * KT_PER_QC + t
                        tsl = slice(t * P, (t + 1) * P)
                        # q projection, natural layout
                        q_ps = psA.tile([P, NH, HD], F32, tag="qps")
                        for kc in range(KC):
                            nc.tensor.matmul(
                                q_ps[:],
                                xt_q[:, kc, tsl],
                                wq_sb[:, kc],
                                start=(kc == 0),
                                stop=(kc == KC - 1),
                            )
                        # kv natural for this tile (fp16 transpose)
                        kvn_ps = psN.tile([P, P], FP16, tag="kvn")
                        nc.tensor.transpose(kvn_ps[:], kvT_f[:, tsl], identh[:])
                        # transposes of the previous tile (PE stays dense)
                        if pending is not None:
                            trans_block(*pending)
                        nc.vector.tensor_copy(v_sb[:, pt, 0:HD], kvn_ps[:, HD:P])
                        # sum-of-squares for q heads + k, sqrt, reciprocal
                        qkss = worka.tile([P, NH + 1], F32, tag="qkss")
                        qsq = worka.tile([P, NH, HD], F32, tag="qsq")
                        nc.scalar.activation(
                            out=qsq[:], in_=q_ps[:],
                            func=mybir.ActivationFunctionType.Square,
                        )
                        nc.vector.reduce_sum(
                            out=qkss[:, 0:NH], in_=qsq[:],
                            axis=mybir.AxisListType.X,
                        )
                        ksq = worka.tile([P, HD], F32, tag="ksq")
                        nc.scalar.activation(
                            out=ksq[:], in_=kvn_ps[:, 0:HD],
                            func=mybir.ActivationFunctionType.Square,
                        )
                        nc.vector.reduce_sum(
                            out=qkss[:, NH : NH + 1], in_=ksq[:],
                            axis=mybir.AxisListType.X,
                        )
                        nc.scalar.activation(
                            out=qkss[:], in_=qkss[:],
                            func=mybir.ActivationFunctionType.Sqrt,
                            bias=eps_sb[:],
                            scale=1.0 / HD,
                        )
                        nc.vector.reciprocal(out=qkss[:], in_=qkss[:])
                        # rope (psum -> fp16), then rms-scale
                        qro = worka.tile([P, NH, HD], FP16, tag="qro")
                        _rope(nc, worka, qro, q_ps[:], cs_sb[:, pt], sc_sb[:, pt], NH)
                        nc.vector.tensor_tensor(
                            qro[:],
                            qro[:],
                            qkss[:, 0:NH, None].to_broadcast([P, NH, HD]),
                            mybir.AluOpType.mult,
                        )
                        kro = worka.tile([P, HD], FP16, tag="kro")
                        _rope(nc, worka, kro, kvn_ps[:, 0:HD], cs_sb[:, pt], sc_sb[:, pt], 1)
                        nc.vector.tensor_scalar_mul(
                            kro[:], kro[:], qkss[:, NH : NH + 1]
                        )
                        pending = (pt, qro, kro)

                    # gate pre-activations (transposed layout), raw fp16
                    for jc in range(NPAIR):
                        g_ps = psG.tile([P, 512], F32, tag="gps")
                        for kc in range(KC):
                            nc.tensor.matmul(
                                g_ps[:],
                                wg_sb[:, kc, jc * P : (jc + 1) * P],
                                xt_q[:, kc],
                                start=(kc == 0),
                                stop=(kc == KC - 1),
                            )
                        nc.vector.tensor_copy(
                            graw[:, jc, qtr * 512 : (qtr + 1) * 512], g_ps[:]
                        )
                # batched sigmoid: quarters 0-2 first (overlaps last transposes),
                # quarter 3 after; one sigmoid table load total
                for jc in range(NPAIR):
                    nc.scalar.activation(
                        out=gateT[:, jc, 0 : 3 * 512],
                        in_=graw[:, jc, 0 : 3 * 512],
                        func=mybir.ActivationFunctionType.Sigmoid,
                    )
                trans_block(*pending)
                for jc in range(NPAIR):
                    nc.scalar.activation(
                        out=gateT[:, jc, 3 * 512 : Lk],
                        in_=graw[:, jc, 3 * 512 : Lk],
                        func=mybir.ActivationFunctionType.Sigmoid,
                    )

            # --------- phase C: attention + fused o_proj (j-outer) ---------
            with (
                tc.tile_pool(name="stp", bufs=2, space="PSUM") as stp,
                tc.tile_pool(name="pvp", bufs=2, space="PSUM") as pvp,
                tc.tile_pool(name="prp", bufs=4) as prp,
                tc.tile_pool(name="ogp", bufs=2) as ogp,
                tc.tile_pool(name="workc", bufs=2) as workc,
                tc.tile_pool(name="ypool", bufs=2) as ypool,
            ):
                rowsA = slice(0, HD)
                rowsB = slice(HD, 2 * HD)
                for j in range(QC):
                    qsl = slice(j * 512, (j + 1) * 512)
                    nkt = KT_PER_QC * (j + 1)
                    outg_j = ogp.tile([P, NPAIR, 512], FP16, tag="ogj")
                    for p in range(NPAIR):
                        hA, hB = 2 * p, 2 * p + 1
                        pvA_ps = pvp.tile([P, 512], F32, tag="pvA")
                        pvB_ps = pvp.tile([P, 512], F32, tag="pvB")
                        prs = {}

                        def do_pv(kt):
                            off = kt - KT_PER_QC * j
                            qlo = off * P if off > 0 else 0
                            pr = prs.pop(kt)
                            nc.tensor.matmul(
                                pvA_ps[:, qlo:512],
                                v_sb[:, kt, :],
                                pr[:, 0, qlo:512],
                                start=(kt == 0),
                                stop=(kt == nkt - 1),
                                skip_group_check=True,
                            )
                            nc.tensor.matmul(
                                pvB_ps[:, qlo:512],
                                v_sb[:, kt, :],
                                pr[:, 1, qlo:512],
                                start=(kt == 0),
                                stop=(kt == nkt - 1),
                                skip_group_check=True,
                            )

                        for kt in range(nkt):
                            off = kt - KT_PER_QC * j
                            qlo = off * P if off > 0 else 0
                            ksl = slice(kt * P, (kt + 1) * P)
                            gqs = slice(j * 512 + qlo, (j + 1) * 512)
                            st = stp.tile([P, 2, 512], F32, tag="st")
                            nc.tensor.matmul(
                                st[:, 0, qlo:512],
                                qkT[:, NH, ksl],
                                qkT[:, hA, gqs],
                                start=True, stop=True,
                                skip_group_check=True,
                            )
                            nc.tensor.matmul(
                                st[:, 1, qlo:512],
                                qkT[:, NH, ksl],
                                qkT[:, hB, gqs],
                                start=True, stop=True,
                                skip_group_check=True,
                            )
                            pr = prp.tile([P, 2, 512], FP16, tag="pr")
                            prs[kt] = pr
                            nc.scalar.activation(
                                out=pr[:, :, qlo:512],
                                in_=st[:, :, qlo:512],
                                func=mybir.ActivationFunctionType.Exp,
                                scale=SCALE,
                            )
                            if off >= 0:
                                nc.vector.tensor_tensor(
                                    pr[:, :, qlo : qlo + P],
                                    pr[:, :, qlo : qlo + P],
                                    mask_sb[:, None, :].to_broadcast([P, 2, P]),
                                    mybir.AluOpType.mult,
                                )
                            if kt >= 2:
                                do_pv(kt - 2)
                        do_pv(nkt - 2)
                        do_pv(nkt - 1)
                        # drain pv psum to SBUF immediately (frees the psum
                        # slots for the next pair / o_proj), then normalize
                        pvs = workc.tile([P, 2, 512], F32, tag="pvs")
                        nc.vector.tensor_copy(pvs[0 : HD + 1, 0], pvA_ps[0 : HD + 1, :])
                        nc.vector.tensor_copy(pvs[0 : HD + 1, 1], pvB_ps[0 : HD + 1, :])
                        smA = workc.tile([1, 512], F32, tag="smA")
                        smB = workc.tile([1, 512], F32, tag="smB")
                        nc.vector.tensor_copy(smA[:], pvs[HD : HD + 1, 0])
                        nc.vector.tensor_copy(smB[:], pvs[HD : HD + 1, 1])
                        recA = workc.tile([1, 512], F32, tag="recA")
                        recB = workc.tile([1, 512], F32, tag="recB")
                        nc.vector.reciprocal_approx_fast(out=recA[:], in_=smA[:])
                        nc.vector.reciprocal_approx_fast(out=recB[:], in_=smB[:])
                        rbgA = workc.tile([HD, 512], F32, tag="rbgA")
                        rbgB = workc.tile([HD, 512], F32, tag="rbgB")
                        nc.gpsimd.partition_broadcast(rbgA[:], recA[:])
                        nc.gpsimd.partition_broadcast(rbgB[:], recB[:])
                        og = workc.tile([P, 512], F32, tag="og")
                        nc.vector.tensor_tensor(
                            og[rowsA, :], pvs[0:HD, 0], rbgA[:],
                            mybir.AluOpType.mult,
                        )
                        nc.vector.tensor_tensor(
                            og[rowsB, :], pvs[0:HD, 1], rbgB[:],
                            mybir.AluOpType.mult,
                        )
                        nc.vector.tensor_tensor(
                            outg_j[:, p], og[:], gateT[:, p, qsl],
                            mybir.AluOpType.mult,
                        )
                    # fused o_proj for this 512-wide q chunk
                    JC = NH * HD // P
                    for t in range(KT_PER_QC):
                        qt = j * KT_PER_QC + t
                        y_sb = ypool.tile([P, Dk], FP16, tag="ysb")
                        for dc in range(Dk // 512):
                            y_ps = pvp.tile([P, 512], F32, tag="pvA")
                            for jc in range(JC):
                                nc.tensor.matmul(
                                    y_ps[:],
                                    outg_j[:, jc, t * P : (t + 1) * P],
                                    wo_sb[:, jc, dc * 512 : (dc + 1) * 512],
                                    start=(jc == 0),
                                    stop=(jc == JC - 1),
                                )
                            nc.vector.tensor_copy(
                                y_sb[:, dc * 512 : (dc + 1) * 512], y_ps[:]
                            )
                        nc.sync.dma_start(y_r[:, qt], y_sb[:])

    nc.compile()
    return nc


def _rope(nc, pool, out, in_, cs_t, sc_t, nh):
    """Split-half rope via packed tables: cs = [cos|sin], sc = [sin|cos].
    ta = in*cs = [x1*cos | x2*sin]; tb = in*sc = [x1*sin | x2*cos];
    out1 = ta1 - ta2; out2 = tb1 + tb2. 4 DVE ops."""
    HALF = HD // 2
    if nh == 1:
        o1 = out[:, 0:HALF]
        o2 = out[:, HALF:HD]
        csb = cs_t
        scb = sc_t
        shape = [P, HD]
        def half(t, i):
            return t[:, i * HALF : (i + 1) * HALF]
    else:
        o1 = out[:, :, 0:HALF]
        o2 = out[:, :, HALF:HD]
        csb = cs_t[:, None, :].to_broadcast([P, nh, HD])
        scb = sc_t[:, None, :].to_broadcast([P, nh, HD])
        shape = [P, nh, HD]
        def half(t, i):
            return t[:, :, i * HALF : (i + 1) * HALF]
    ta = pool.tile(shape, F32, tag="rope_a")
    tb = pool.tile(shape, F32, tag="rope_b")
    nc.vector.tensor_tensor(ta[:], in_, csb, mybir.AluOpType.mult)
    nc.vector.tensor_tensor(tb[:], in_, scb, mybir.AluOpType.mult)
    nc.vector.tensor_tensor(o1, half(ta, 0), half(ta, 1), mybir.AluOpType.subtract)
    nc.vector.tensor_tensor(o2, half(tb, 0), half(tb, 1), mybir.AluOpType.add)


def _host_inputs(x, Wq, Wk, Wv, Wg, Wo, Lk=L, Dk=D):
    """Build the 8 per-core input maps."""
    half = HD // 2
    inv_freq = 1.0 / (THETA ** (np.arange(0, half, dtype=np.float64) / half))
    ang = np.arange(Lk, dtype=np.float64)[:, None] * inv_freq[None, :]
    cos_t = np.cos(ang).astype(np.float32)
    sin_t = np.sin(ang).astype(np.float32)

    kk = np.arange(P)[:, None]
    qq = np.arange(P)[None, :]
    mask = (qq >= kk).astype(np.float16)

    in_maps = []
    for c in range(8):
        b, g = c // 4, c % 4
        xT = np.ascontiguousarray(x[b].T)
        in_maps.append(
            {
                "xt": xT.astype(np.float16),
                "wq": np.ascontiguousarray(Wq[:, g * NH * HD : (g + 1) * NH * HD]).astype(np.float16),
                "wkv": np.ascontiguousarray(
                    np.concatenate(
                        [
                            Wk[:, g * HD : (g + 1) * HD],
                            Wv[:, g * HD : (g + 1) * HD],
                        ],
                        axis=1,
                    )
                ).astype(np.float16),
                "wg": np.ascontiguousarray(Wg[:, g * NH * HD : (g + 1) * NH * HD]).astype(np.float16),
                "wo": np.ascontiguousarray(Wo[g * NH * HD : (g + 1) * NH * HD, :]).astype(np.float16),
                "cos": cos_t,
                "sin": sin_t,
                "mask": mask,
            }
        )
    return in_maps


_CACHED = {}


def kernel(x, Wq, Wk, Wv, Wg, Wo, qn_w, kn_w, mask, _trace=False, _tmpdir=None):
    """Full-input entry point. Returns [B, L, D] float32."""
    if "nc" not in _CACHED:
        _CACHED["nc"] = build_core_kernel()
    nc = _CACHED["nc"]
    in_maps = _host_inputs(
        np.asarray(x), np.asarray(Wq), np.asarray(Wk), np.asarray(Wv),
        np.asarray(Wg), np.asarray(Wo),
    )
    res = run_bass_kernel_spmd(
        nc, in_maps, core_ids=list(range(8)), trace=_trace, tmpdir=_tmpdir
    )
    out = np.zeros((B, L, D), dtype=np.float32)
    for c in range(8):
        out[c // 4] += res.results[c]["y"].astype(np.float32)
    if _trace:
        kernel.last_exec_time_ns = res.exec_time_ns
    return out
